# revision 13
# baseline (speedup 1.0000x reference)
"""DynamicSoftKMeansLoss on 8 Trainium2 NeuronCores.

Strategy (data-parallel over B, hardcoded for B=200000, D=256, K=5, C=16):
  - The loss depends on feat rows ONLY where label2==1 (every segment sum is
    w-weighted; the presence/count terms are host-side bincounts), so the
    host filters to those ~B/2 rows first — halving all device work.
  - Host pads the filtered rows to 8*T*128, shards across 8 cores,
    pre-transposes each shard to partition-major XT [128 dpart, 2 dchunk,
    T tiles, 128 rows] and casts to fp8 e4m3 (x scaled by 16, centers by
    1/16 so the PE product is unscaled). Host also precomputes the
    w-weighted one-hot labels (bf16, padded rows all-zero). feat_normed has
    unit rows by construction, so |x|^2 = 1 and d2 = 1 + |c_k|^2 - 2 x.c
    needs only a [K]-vector add (host-verified, with a per-row-table
    fallback build if rows are not unit-norm).
  - Per G-batch on device (sizes 7/14/21/28/28... so compute starts before
    the full x stream lands): 2 fp8 matmuls per tile -> -2 x.c in PSUM;
    dist = exp(0.5*ln(psum + d2add)) written straight into vals as bf16;
    softmax weighted dist wd; min/2nd-min over the 5 centers. Instead of
    materializing viol_j per row, vals carries v1 = relu(wd+m-min) and
    mask_j*(v2-v1) so the per-class argmin slot is resolved on the host:
    sum viol_{j*} = sum v1 + [sum mask_j*(v2-v1)]_{j*}.
    The loop is software-pipelined: batch b+1's PSUM add + Ln/Exp run ahead
    of batch b's DVE chain; dummy warmup matmuls ramp the PE pstate early.
  - All per-class reductions are packed accumulating matmuls:
    psum[84,112] += vals7^T @ oh7 where 7 tiles' [128,12] vals / [128,16]
    one-hots are fused into one PE op (12 metrics: dist(5) | v1 |
    mask*dv(5) | wd^2).
  - Each core DMAs its [84,112] partial to DRAM; host sums the 8 partials,
    extracts the 7 diagonal [12,16] blocks, and does the tiny per-class
    argmin + final reduction in numpy (replaces a ~55us on-device
    collective+final-stage tail).
"""

import sys

sys.path.insert(0, "/opt/trn_rl_repo")

import numpy as np

import concourse.bass as bass
import concourse.bacc as bacc
import concourse.tile as tile
from concourse import mybir
from concourse.bass_utils import run_bass_kernel_spmd

F32 = mybir.dt.float32
BF16 = mybir.dt.bfloat16
F8 = mybir.dt.float8e4
ALU = mybir.AluOpType
ACTF = mybir.ActivationFunctionType
AX = mybir.AxisListType

B, D, K, C = 200000, 256, 5, 16
NCORES = 8
MARGIN = 0.5
BIG = float(2.0**40)

NM = 12              # vals metrics: dist(5) | v1 | mask*dv(5) | wd^2
SEGP = 7             # tiles packed per segment matmul
XSCALE = 16.0        # fp8 range centering: x*16 on host, centers/16
NWARM = 48           # PE pstate warmup matmuls


def _batches(tiles):
    """Split tiles into G-batches (multiples of SEGP, ramped sizes so the
    first DVE chains start before the full x stream lands)."""
    assert tiles % SEGP == 0
    bs = []
    rem = tiles
    for want in (7, 14, 21):
        if rem <= 0:
            break
        c = min(want, rem)
        bs.append(c)
        rem -= c
    while rem:
        c = min(28, rem)
        bs.append(c)
        rem -= c
    return bs


def _b0(ap, n, axis="inner"):
    """Stride-0 broadcast of a 2D [128, G] AP to 3D."""
    pairs = [list(p) for p in ap.ap]
    if axis == "inner":
        newap = pairs + [[0, n]]
    else:
        newap = [pairs[0], [0, n], pairs[1]]
    return bass.AP(tensor=ap.tensor, offset=ap.offset, ap=newap)


def _patch_act_tables():
    """Placement-only hint: hide Ln/Exp from every table except the combined
    natural_log_exp_and_others so Bacc's greedy table-load placement picks the
    one table that serves Ln and Exp together (ids stay valid)."""
    import concourse.bacc as _bacc
    from concourse.hw_specs import get_activation_tables as _orig

    def patched(arch):
        tabs = _orig(arch)
        keep = "natural_log_exp_and_others"
        if keep in tabs:
            for name, funcs in tabs.items():
                if name != keep:
                    funcs.discard(ACTF.Ln)
                    funcs.discard(ACTF.Exp)
        return tabs

    _bacc.get_activation_tables = patched


def build_nc(tiles, unit_norm, n_cores=NCORES):
    _patch_act_tables()
    nc = bacc.Bacc(None, num_devices=n_cores)
    gbs = _batches(tiles)
    nb = len(gbs)

    x_dram = nc.declare_dram_parameter("x", [128, 2, tiles, 128], F8, isOutput=False)
    ncst = K if unit_norm else tiles * K
    const_dram = nc.declare_dram_parameter("const", [128, ncst], F32, isOutput=False)
    oh_dram = nc.declare_dram_parameter("oh", [128, tiles, C], BF16, isOutput=False)
    cf8_dram = nc.declare_dram_parameter("cf8", [128, 2 * K], F8, isOutput=False)
    out_dram = nc.declare_dram_parameter(
        "out", [SEGP * NM, SEGP * C], F32, isOutput=True
    )

    with tile.TileContext(nc) as tc:
        with (
            tc.tile_pool(name="consts", bufs=1) as consts,
            tc.tile_pool(name="xin", bufs=4) as xin,
            tc.tile_pool(name="ohin", bufs=3) as ohin,
            tc.tile_pool(name="big", bufs=2) as big,
            tc.tile_pool(name="stat", bufs=2) as stat,
            tc.tile_pool(name="ps_d", bufs=2, space="PSUM") as psd_pool,
            tc.tile_pool(name="ps_seg", bufs=1, space="PSUM") as psseg,
            tc.tile_pool(name="ps_warm", bufs=1, space="PSUM") as pswarm,
        ):
            cf8_sb = consts.tile([128, 2 * K], F8)
            nc.sync.dma_start(cf8_sb[:], cf8_dram[:])
            const_sb = consts.tile([128, ncst], F32)
            nc.scalar.dma_start(const_sb[:], const_dram[:])

            # PE pstate warmup: keep the tensor engine continuously busy from
            # right after the tiny cf8 load until real tiles arrive
            warm_ps = pswarm.tile([2 * K, 2 * K], F32)
            for _ in range(NWARM):
                nc.tensor.matmul(
                    warm_ps[:], cf8_sb[:], cf8_sb[:], start=True, stop=True
                )

            psum_seg = psseg.tile([SEGP * NM, SEGP * C], F32)

            offs = np.cumsum([0] + gbs)
            st = [dict() for _ in range(nb)]

            def emit_load_mm(b):
                gb, t0, t1 = gbs[b], offs[b], offs[b + 1]
                xb = xin.tile([128, 2, gb, 128], F8, tag="xb")
                nc.sync.dma_start(xb[:, 0], x_dram[:, 0, t0:t1, :])
                nc.sync.dma_start(xb[:, 1], x_dram[:, 1, t0:t1, :])
                oh = ohin.tile([128, gb, C], BF16, tag="oh")
                nc.scalar.dma_start(oh[:], oh_dram[:, t0:t1, :])
                psd = psd_pool.tile([128, gb, K], F32, tag="psd")
                for c in range(2):
                    for g in range(gb):
                        nc.tensor.matmul(
                            psd[:, g, :], xb[:, c, g, :],
                            cf8_sb[:, c * K:(c + 1) * K],
                            start=(c == 0), stop=(c == 1),
                        )
                st[b]["psd"], st[b]["oh"] = psd, oh

            def emit_a(b):
                """PSUM readout + ACT chain for batch b (runs ahead of batch
                b-1's DVE chain)."""
                gb, t0, t1 = gbs[b], offs[b], offs[b + 1]
                t_d2 = big.tile([128, gb, K], F32, tag="t_d2")
                if unit_norm:
                    d2add = _b0(const_sb[:, 0:K], gb, "outer")
                else:
                    d2add = const_sb[:, t0 * K:t1 * K].rearrange(
                        "p (g k) -> p g k", k=K
                    )
                nc.vector.tensor_tensor(t_d2[:], st[b]["psd"][:], d2add, ALU.add)
                lnt = big.tile([128, gb, K], F32, tag="lnt")
                nc.scalar.activation(lnt[:], t_d2[:], ACTF.Ln)
                vals = big.tile([128, gb, NM], BF16, tag="vals")
                nc.scalar.activation(vals[:, :, 0:K], lnt[:], ACTF.Exp, scale=0.5)
                eu = big.tile([128, gb, K], BF16, tag="eu")
                nc.scalar.activation(eu[:], vals[:, :, 0:K], ACTF.Exp, scale=-1.0)
                prod = big.tile([128, gb, K], BF16, tag="prod")
                nc.gpsimd.tensor_tensor(prod[:], eu[:], vals[:, :, 0:K], ALU.mult)
                st[b]["vals"], st[b]["eu"], st[b]["prod"] = vals, eu, prod

            def emit_b(b):
                """Main DVE chain + packed segment matmuls for batch b."""
                gb = gbs[b]
                vals, eu, oh = st[b]["vals"], st[b]["eu"], st[b]["oh"]
                prod = st[b]["prod"]
                dist = vals[:, :, 0:K]

                m1 = stat.tile([128, gb], BF16, tag="m1")
                nc.vector.tensor_reduce(m1[:], dist, axis=AX.X, op=ALU.min)
                mask = big.tile([128, gb, K], BF16, tag="mask")
                nc.vector.tensor_tensor(
                    mask[:], dist, _b0(m1[:], K), ALU.is_equal
                )
                dmask = big.tile([128, gb, K], BF16, tag="dmask")
                nc.vector.scalar_tensor_tensor(
                    dmask[:], mask[:], BIG, dist, ALU.mult, ALU.add
                )
                m2 = stat.tile([128, gb], BF16, tag="m2")
                nc.vector.tensor_reduce(m2[:], dmask[:], axis=AX.X, op=ALU.min)

                s = stat.tile([128, gb], F32, tag="s")
                nc.vector.tensor_reduce(s[:], eu[:], axis=AX.X, op=ALU.add)
                spd = stat.tile([128, gb], F32, tag="spd")
                nc.vector.tensor_reduce(spd[:], prod[:], axis=AX.X, op=ALU.add)
                rs = stat.tile([128, gb], F32, tag="rs")
                nc.vector.reciprocal(rs[:], s[:])
                wd = stat.tile([128, gb], F32, tag="wd")
                nc.vector.tensor_tensor(wd[:], spd[:], rs[:], ALU.mult)

                wd3 = wd[:].rearrange("p (g o) -> p g o", o=1)
                nc.scalar.activation(vals[:, :, 11:12], wd3, ACTF.Square)

                # v1 = relu(wd+m-m1) -> vals[5]; v2 = relu(wd+m-m2);
                # vals[6:11] = mask * (v2 - v1)
                t1s = stat.tile([128, gb], F32, tag="t1")
                nc.vector.scalar_tensor_tensor(
                    t1s[:], wd[:], MARGIN, m1[:], ALU.add, ALU.subtract
                )
                t2s = stat.tile([128, gb], F32, tag="t2")
                nc.vector.scalar_tensor_tensor(
                    t2s[:], wd[:], MARGIN, m2[:], ALU.add, ALU.subtract
                )
                t13 = t1s[:].rearrange("p (g o) -> p g o", o=1)
                nc.scalar.activation(vals[:, :, K:K + 1], t13, ACTF.Relu)
                v2 = stat.tile([128, gb], F32, tag="v2")
                nc.scalar.activation(v2[:], t2s[:], ACTF.Relu)
                dvv = stat.tile([128, gb], F32, tag="dvv")
                nc.vector.tensor_tensor(
                    dvv[:], v2[:],
                    vals[:, :, K:K + 1].rearrange("p g o -> p (g o)"),
                    ALU.subtract,
                )
                nc.vector.tensor_tensor(
                    vals[:, :, 6:11], mask[:], _b0(dvv[:], K), ALU.mult
                )

                npk = gb // SEGP
                p_base = offs[b] // SEGP
                for p in range(npk):
                    nc.tensor.matmul(
                        psum_seg[:],
                        vals[:, p * SEGP:(p + 1) * SEGP, :].rearrange(
                            "p g m -> p (g m)"
                        ),
                        oh[:, p * SEGP:(p + 1) * SEGP, :].rearrange(
                            "p g c -> p (g c)"
                        ),
                        start=(p_base + p == 0),
                        stop=(p_base + p == tiles // SEGP - 1),
                    )

            # software pipeline: A(b+1) is emitted before B(b) so the PSUM
            # readout + Ln/Exp of the next batch overlap the current DVE chain
            emit_load_mm(0)
            emit_a(0)
            for b in range(nb):
                if b + 1 < nb:
                    emit_load_mm(b + 1)
                    emit_a(b + 1)
                emit_b(b)

            seg_sb = consts.tile([SEGP * NM, SEGP * C], F32, tag="seg_sb")
            nc.vector.tensor_copy(seg_sb[:], psum_seg[:])
            nc.sync.dma_start(out_dram[:], seg_sb[:])

    nc.compile()
    return nc


def _host_prep(feat, labels, label2, centers, tiles, unit_norm, n_cores=NCORES):
    """Filter w==1 rows, pad + shard + pre-transpose + fp8-cast."""
    import ml_dtypes

    rpc = tiles * 128
    bpad = rpc * n_cores

    feat = np.asarray(feat, dtype=np.float32)
    labels = np.asarray(labels)
    label2 = np.asarray(label2)
    centers = np.asarray(centers, dtype=np.float32)

    idx = np.flatnonzero(label2 == 1)
    nw = idx.size

    xpad = np.zeros((bpad, D), dtype=np.float32)
    xpad[:nw] = feat[idx]
    x_f8 = (xpad * XSCALE).astype(ml_dtypes.float8_e4m3)

    wlab = np.full(bpad, C, dtype=np.int64)
    wlab[:nw] = labels[idx]
    oh_full = (wlab[:, None] == np.arange(C)[None, :]).astype(ml_dtypes.bfloat16)

    ctilT = (centers.T * (-2.0 / XSCALE)).astype(ml_dtypes.float8_e4m3)
    cf8 = np.ascontiguousarray(
        np.concatenate([ctilT[0:128], ctilT[128:256]], axis=1)
    )  # [128, 10]
    cnorm = (centers * centers).sum(axis=1).astype(np.float32)  # [5]

    if not unit_norm:
        norm2 = np.einsum("ij,ij->i", xpad, xpad, dtype=np.float32)

    in_maps = []
    for i in range(n_cores):
        sl = slice(i * rpc, (i + 1) * rpc)
        xi = np.ascontiguousarray(
            x_f8[sl].reshape(tiles, 128, 2, 128).transpose(3, 2, 0, 1)
        )
        if unit_norm:
            d2add = np.ascontiguousarray(
                np.tile((cnorm + 1.0)[None, :], (128, 1))
            )
        else:
            n2 = norm2[sl].reshape(tiles, 128).T  # [128, tiles]
            d2add = np.ascontiguousarray(
                (n2[:, :, None] + cnorm[None, None, :]).reshape(128, tiles * K)
            )
        ohi = np.ascontiguousarray(
            oh_full[sl].reshape(tiles, 128, C).transpose(1, 0, 2)
        )
        in_maps.append({"x": xi, "const": d2add, "oh": ohi, "cf8": cf8})
    return in_maps


def _host_final(parts, labels, label2, num_classes):
    """Sum per-core [84,112] partials, extract diagonal [12,16] blocks, and
    do the per-class argmin + final reduction (mirrors the reference)."""
    seg = np.zeros((NM, C), dtype=np.float64)
    if parts:
        S = np.zeros((SEGP * NM, SEGP * C), dtype=np.float64)
        for p in parts:
            S += np.asarray(p, dtype=np.float64)
        for p in range(SEGP):
            seg += S[p * NM:(p + 1) * NM, p * C:(p + 1) * C]

    labels = np.asarray(labels).astype(np.int64)
    label2 = np.asarray(label2)
    Ci = int(num_classes)
    w = (label2 == 1)
    cnt = np.bincount(labels[w], minlength=Ci).astype(np.float64)[:C]
    present = np.bincount(labels, minlength=Ci)[:C] > 0

    safe = np.maximum(cnt, 1.0)
    meand = seg[0:K] / safe[None, :]          # [K, C]
    closest = np.argmin(meand, axis=0)        # [C]
    # sum_c viol = sum v1 + [sum mask_j*(v2-v1)] at the class's closest j
    sv = seg[K] + seg[K + 1 + closest, np.arange(C)]
    has = (cnt > 0).astype(np.float64)
    per_class = (seg[11] + sv) / safe * has
    n_unique = max(float(present.sum()), 1.0)
    return np.float32(per_class.sum() / n_unique)


_NC_CACHE = {}


def kernel(feat_normed, labels, label2, num_classes, centers, _trace=False):
    label2 = np.asarray(label2)
    nw = int((label2 == 1).sum())
    if nw == 0:
        return np.asarray(
            _host_final([], labels, label2, num_classes), dtype=np.float32
        )
    feat_normed = np.asarray(feat_normed, dtype=np.float32)
    # unit-norm fast path (feat_normed is normalized by construction);
    # sampled check with a per-row-|x|^2-table fallback build
    samp = feat_normed[:: max(1, feat_normed.shape[0] // 512)]
    unit_norm = bool(
        np.allclose(np.einsum("ij,ij->i", samp, samp), 1.0, atol=1e-3)
    )
    tiles = -(-nw // (128 * NCORES))          # ceil rows / (128*cores)
    tiles = SEGP * (-(-tiles // SEGP))        # round up to multiple of SEGP
    key = (tiles, unit_norm)
    if key not in _NC_CACHE:
        _NC_CACHE[key] = build_nc(tiles, unit_norm)
    nc = _NC_CACHE[key]
    in_maps = _host_prep(
        feat_normed, labels, label2, centers, tiles, unit_norm
    )
    res = run_bass_kernel_spmd(
        nc, in_maps, core_ids=list(range(NCORES)), trace=_trace
    )
    parts = [r["out"] for r in res.results]
    out = _host_final(parts, labels, label2, num_classes)
    if _trace:
        kernel.last_result = res
    return np.asarray(out, dtype=np.float32)


# revision 14
# speedup vs baseline: 1.0008x; 1.0008x over previous
"""DynamicSoftKMeansLoss on 8 Trainium2 NeuronCores.

Strategy (data-parallel over B, hardcoded for B=200000, D=256, K=5, C=16):
  - The loss depends on feat rows ONLY where label2==1 (every segment sum is
    w-weighted; the presence/count terms are host-side bincounts), so the
    host filters to those ~B/2 rows first — halving all device work.
  - Host pads the filtered rows to 8*T*128, shards across 8 cores,
    pre-transposes each shard to partition-major XT [128 dpart, 2 dchunk,
    T tiles, 128 rows] and casts to fp8 e4m3 (x scaled by 16, centers by
    1/16 so the PE product is unscaled). Host also precomputes the
    w-weighted one-hot labels (bf16, padded rows all-zero). feat_normed has
    unit rows by construction, so |x|^2 = 1 and d2 = 1 + |c_k|^2 - 2 x.c
    needs only a [K]-vector add (host-verified, with a per-row-table
    fallback build if rows are not unit-norm).
  - Per G-batch on device (sizes 7/14/21/28/28... so compute starts before
    the full x stream lands): 2 fp8 matmuls per tile -> -2 x.c in PSUM;
    dist = exp(0.5*ln(psum + d2add)) written straight into vals as bf16;
    softmax weighted dist wd; min/2nd-min over the 5 centers. Instead of
    materializing viol_j per row, vals carries v1 = relu(wd+m-min) and
    mask_j*(v2-v1) so the per-class argmin slot is resolved on the host:
    sum viol_{j*} = sum v1 + [sum mask_j*(v2-v1)]_{j*}.
    The loop is software-pipelined: batch b+1's PSUM add + Ln/Exp run ahead
    of batch b's DVE chain; dummy warmup matmuls ramp the PE pstate early.
  - All per-class reductions are packed accumulating matmuls:
    psum[84,112] += vals7^T @ oh7 where 7 tiles' [128,12] vals / [128,16]
    one-hots are fused into one PE op (12 metrics: dist(5) | v1 |
    mask*dv(5) | wd^2).
  - Each core DMAs its [84,112] partial to DRAM; host sums the 8 partials,
    extracts the 7 diagonal [12,16] blocks, and does the tiny per-class
    argmin + final reduction in numpy (replaces a ~55us on-device
    collective+final-stage tail).
"""

import sys

sys.path.insert(0, "/opt/trn_rl_repo")

import numpy as np

import concourse.bass as bass
import concourse.bacc as bacc
import concourse.tile as tile
from concourse import mybir
from concourse.bass_utils import run_bass_kernel_spmd

F32 = mybir.dt.float32
BF16 = mybir.dt.bfloat16
F8 = mybir.dt.float8e4
ALU = mybir.AluOpType
ACTF = mybir.ActivationFunctionType
AX = mybir.AxisListType

B, D, K, C = 200000, 256, 5, 16
NCORES = 8
MARGIN = 0.5
BIG = float(2.0**40)

NM = 12              # vals metrics: dist(5) | v1 | mask*dv(5) | wd^2
SEGP = 7             # tiles packed per segment matmul
XSCALE = 16.0        # fp8 range centering: x*16 on host, centers/16
NWARM = 48           # PE pstate warmup matmuls


def _batches(tiles):
    """Split tiles into G-batches (multiples of SEGP, ramped sizes so the
    first DVE chains start before the full x stream lands)."""
    assert tiles % SEGP == 0
    bs = []
    rem = tiles
    for want in (7, 14, 21):
        if rem <= 0:
            break
        c = min(want, rem)
        bs.append(c)
        rem -= c
    while rem:
        c = min(28, rem)
        bs.append(c)
        rem -= c
    return bs


def _b0(ap, n, axis="inner"):
    """Stride-0 broadcast of a 2D [128, G] AP to 3D."""
    pairs = [list(p) for p in ap.ap]
    if axis == "inner":
        newap = pairs + [[0, n]]
    else:
        newap = [pairs[0], [0, n], pairs[1]]
    return bass.AP(tensor=ap.tensor, offset=ap.offset, ap=newap)


def _patch_act_tables():
    """Placement-only hint: hide Ln/Exp from every table except the combined
    natural_log_exp_and_others so Bacc's greedy table-load placement picks the
    one table that serves Ln and Exp together (ids stay valid)."""
    import concourse.bacc as _bacc
    from concourse.hw_specs import get_activation_tables as _orig

    def patched(arch):
        tabs = _orig(arch)
        keep = "natural_log_exp_and_others"
        if keep in tabs:
            for name, funcs in tabs.items():
                if name != keep:
                    funcs.discard(ACTF.Ln)
                    funcs.discard(ACTF.Exp)
        return tabs

    _bacc.get_activation_tables = patched


def build_nc(tiles, unit_norm, n_cores=NCORES):
    _patch_act_tables()
    nc = bacc.Bacc(None, num_devices=n_cores)
    gbs = _batches(tiles)
    nb = len(gbs)

    x_dram = nc.declare_dram_parameter("x", [128, 2, tiles, 128], F8, isOutput=False)
    ncst = K if unit_norm else tiles * K
    const_dram = nc.declare_dram_parameter("const", [128, ncst], F32, isOutput=False)
    oh_dram = nc.declare_dram_parameter("oh", [128, tiles, C], BF16, isOutput=False)
    cf8_dram = nc.declare_dram_parameter("cf8", [128, 2 * K], F8, isOutput=False)
    out_dram = nc.declare_dram_parameter(
        "out", [SEGP * NM, SEGP * C], F32, isOutput=True
    )

    with tile.TileContext(nc) as tc:
        with (
            tc.tile_pool(name="consts", bufs=1) as consts,
            tc.tile_pool(name="xin", bufs=4) as xin,
            tc.tile_pool(name="ohin", bufs=3) as ohin,
            tc.tile_pool(name="big", bufs=3) as big,
            tc.tile_pool(name="stat", bufs=3) as stat,
            tc.tile_pool(name="ps_d", bufs=4, space="PSUM") as psd_pool,
            tc.tile_pool(name="ps_seg", bufs=1, space="PSUM") as psseg,
            tc.tile_pool(name="ps_warm", bufs=1, space="PSUM") as pswarm,
        ):
            cf8_sb = consts.tile([128, 2 * K], F8)
            nc.sync.dma_start(cf8_sb[:], cf8_dram[:])
            const_sb = consts.tile([128, ncst], F32)
            nc.scalar.dma_start(const_sb[:], const_dram[:])

            # PE pstate warmup: keep the tensor engine continuously busy from
            # right after the tiny cf8 load until real tiles arrive
            warm_ps = pswarm.tile([2 * K, 2 * K], F32)
            for _ in range(NWARM):
                nc.tensor.matmul(
                    warm_ps[:], cf8_sb[:], cf8_sb[:], start=True, stop=True
                )

            psum_seg = psseg.tile([SEGP * NM, SEGP * C], F32)

            offs = np.cumsum([0] + gbs)
            st = [dict() for _ in range(nb)]

            def emit_load_mm(b):
                gb, t0, t1 = gbs[b], offs[b], offs[b + 1]
                xb = xin.tile([128, 2, gb, 128], F8, tag="xb")
                nc.sync.dma_start(xb[:, 0], x_dram[:, 0, t0:t1, :])
                nc.sync.dma_start(xb[:, 1], x_dram[:, 1, t0:t1, :])
                oh = ohin.tile([128, gb, C], BF16, tag="oh")
                nc.scalar.dma_start(oh[:], oh_dram[:, t0:t1, :])
                psd = psd_pool.tile([128, gb, K], F32, tag="psd")
                for c in range(2):
                    for g in range(gb):
                        nc.tensor.matmul(
                            psd[:, g, :], xb[:, c, g, :],
                            cf8_sb[:, c * K:(c + 1) * K],
                            start=(c == 0), stop=(c == 1),
                        )
                st[b]["psd"], st[b]["oh"] = psd, oh

            def emit_a(b):
                """PSUM readout + ACT chain for batch b (runs ahead of batch
                b-1's DVE chain)."""
                gb, t0, t1 = gbs[b], offs[b], offs[b + 1]
                t_d2 = big.tile([128, gb, K], F32, tag="t_d2")
                if unit_norm:
                    d2add = _b0(const_sb[:, 0:K], gb, "outer")
                else:
                    d2add = const_sb[:, t0 * K:t1 * K].rearrange(
                        "p (g k) -> p g k", k=K
                    )
                nc.vector.tensor_tensor(t_d2[:], st[b]["psd"][:], d2add, ALU.add)
                lnt = big.tile([128, gb, K], F32, tag="lnt")
                nc.scalar.activation(lnt[:], t_d2[:], ACTF.Ln)
                vals = big.tile([128, gb, NM], BF16, tag="vals")
                nc.scalar.activation(vals[:, :, 0:K], lnt[:], ACTF.Exp, scale=0.5)
                eu = big.tile([128, gb, K], BF16, tag="eu")
                nc.scalar.activation(eu[:], vals[:, :, 0:K], ACTF.Exp, scale=-1.0)
                prod = big.tile([128, gb, K], BF16, tag="prod")
                nc.gpsimd.tensor_tensor(prod[:], eu[:], vals[:, :, 0:K], ALU.mult)
                st[b]["vals"], st[b]["eu"], st[b]["prod"] = vals, eu, prod

            def emit_b(b):
                """Main DVE chain + packed segment matmuls for batch b."""
                gb = gbs[b]
                vals, eu, oh = st[b]["vals"], st[b]["eu"], st[b]["oh"]
                prod = st[b]["prod"]
                dist = vals[:, :, 0:K]

                m1 = stat.tile([128, gb], BF16, tag="m1")
                nc.vector.tensor_reduce(m1[:], dist, axis=AX.X, op=ALU.min)
                mask = big.tile([128, gb, K], BF16, tag="mask")
                nc.vector.tensor_tensor(
                    mask[:], dist, _b0(m1[:], K), ALU.is_equal
                )
                dmask = big.tile([128, gb, K], BF16, tag="dmask")
                nc.vector.scalar_tensor_tensor(
                    dmask[:], mask[:], BIG, dist, ALU.mult, ALU.add
                )
                m2 = stat.tile([128, gb], BF16, tag="m2")
                nc.vector.tensor_reduce(m2[:], dmask[:], axis=AX.X, op=ALU.min)

                s = stat.tile([128, gb], F32, tag="s")
                nc.vector.tensor_reduce(s[:], eu[:], axis=AX.X, op=ALU.add)
                spd = stat.tile([128, gb], F32, tag="spd")
                nc.vector.tensor_reduce(spd[:], prod[:], axis=AX.X, op=ALU.add)
                rs = stat.tile([128, gb], F32, tag="rs")
                nc.vector.reciprocal(rs[:], s[:])
                wd = stat.tile([128, gb], F32, tag="wd")
                nc.vector.tensor_tensor(wd[:], spd[:], rs[:], ALU.mult)

                wd3 = wd[:].rearrange("p (g o) -> p g o", o=1)
                nc.vector.tensor_tensor(vals[:, :, 11:12], wd3, wd3, ALU.mult)

                # v1 = relu(wd+m-m1) -> vals[5]; v2 = relu(wd+m-m2);
                # vals[6:11] = mask * (v2 - v1)
                t1s = stat.tile([128, gb], F32, tag="t1")
                nc.vector.scalar_tensor_tensor(
                    t1s[:], wd[:], MARGIN, m1[:], ALU.add, ALU.subtract
                )
                t2s = stat.tile([128, gb], F32, tag="t2")
                nc.vector.scalar_tensor_tensor(
                    t2s[:], wd[:], MARGIN, m2[:], ALU.add, ALU.subtract
                )
                t13 = t1s[:].rearrange("p (g o) -> p g o", o=1)
                nc.vector.tensor_scalar(
                    vals[:, :, K:K + 1], t13, 0.0, None, ALU.max
                )
                v2 = stat.tile([128, gb], F32, tag="v2")
                nc.vector.tensor_scalar(v2[:], t2s[:], 0.0, None, ALU.max)
                dvv = stat.tile([128, gb], F32, tag="dvv")
                nc.vector.tensor_tensor(
                    dvv[:], v2[:],
                    vals[:, :, K:K + 1].rearrange("p g o -> p (g o)"),
                    ALU.subtract,
                )
                nc.vector.tensor_tensor(
                    vals[:, :, 6:11], mask[:], _b0(dvv[:], K), ALU.mult
                )

            def emit_seg(b):
                gb = gbs[b]
                vals, oh = st[b]["vals"], st[b]["oh"]
                npk = gb // SEGP
                p_base = offs[b] // SEGP
                for p in range(npk):
                    nc.tensor.matmul(
                        psum_seg[:],
                        vals[:, p * SEGP:(p + 1) * SEGP, :].rearrange(
                            "p g m -> p (g m)"
                        ),
                        oh[:, p * SEGP:(p + 1) * SEGP, :].rearrange(
                            "p g c -> p (g c)"
                        ),
                        start=(p_base + p == 0),
                        stop=(p_base + p == tiles // SEGP - 1),
                    )

            # software pipeline: A(b+1) is emitted before B(b) so the PSUM
            # readout + Ln/Exp of the next batch overlap the current DVE chain
            emit_load_mm(0)
            emit_a(0)
            if nb > 1:
                emit_load_mm(1)
                emit_a(1)
            for b in range(nb):
                emit_b(b)
                if b + 2 < nb:
                    emit_load_mm(b + 2)
                    emit_a(b + 2)
                emit_seg(b)

            seg_sb = consts.tile([SEGP * NM, SEGP * C], F32, tag="seg_sb")
            nc.vector.tensor_copy(seg_sb[:], psum_seg[:])
            nc.sync.dma_start(out_dram[:], seg_sb[:])

    nc.compile()
    return nc


def _host_prep(feat, labels, label2, centers, tiles, unit_norm, n_cores=NCORES):
    """Filter w==1 rows, pad + shard + pre-transpose + fp8-cast."""
    import ml_dtypes

    rpc = tiles * 128
    bpad = rpc * n_cores

    feat = np.asarray(feat, dtype=np.float32)
    labels = np.asarray(labels)
    label2 = np.asarray(label2)
    centers = np.asarray(centers, dtype=np.float32)

    idx = np.flatnonzero(label2 == 1)
    nw = idx.size

    xpad = np.zeros((bpad, D), dtype=np.float32)
    xpad[:nw] = feat[idx]
    x_f8 = (xpad * XSCALE).astype(ml_dtypes.float8_e4m3)

    wlab = np.full(bpad, C, dtype=np.int64)
    wlab[:nw] = labels[idx]
    oh_full = (wlab[:, None] == np.arange(C)[None, :]).astype(ml_dtypes.bfloat16)

    ctilT = (centers.T * (-2.0 / XSCALE)).astype(ml_dtypes.float8_e4m3)
    cf8 = np.ascontiguousarray(
        np.concatenate([ctilT[0:128], ctilT[128:256]], axis=1)
    )  # [128, 10]
    cnorm = (centers * centers).sum(axis=1).astype(np.float32)  # [5]

    if not unit_norm:
        norm2 = np.einsum("ij,ij->i", xpad, xpad, dtype=np.float32)

    in_maps = []
    for i in range(n_cores):
        sl = slice(i * rpc, (i + 1) * rpc)
        xi = np.ascontiguousarray(
            x_f8[sl].reshape(tiles, 128, 2, 128).transpose(3, 2, 0, 1)
        )
        if unit_norm:
            d2add = np.ascontiguousarray(
                np.tile((cnorm + 1.0)[None, :], (128, 1))
            )
        else:
            n2 = norm2[sl].reshape(tiles, 128).T  # [128, tiles]
            d2add = np.ascontiguousarray(
                (n2[:, :, None] + cnorm[None, None, :]).reshape(128, tiles * K)
            )
        ohi = np.ascontiguousarray(
            oh_full[sl].reshape(tiles, 128, C).transpose(1, 0, 2)
        )
        in_maps.append({"x": xi, "const": d2add, "oh": ohi, "cf8": cf8})
    return in_maps


def _host_final(parts, labels, label2, num_classes):
    """Sum per-core [84,112] partials, extract diagonal [12,16] blocks, and
    do the per-class argmin + final reduction (mirrors the reference)."""
    seg = np.zeros((NM, C), dtype=np.float64)
    if parts:
        S = np.zeros((SEGP * NM, SEGP * C), dtype=np.float64)
        for p in parts:
            S += np.asarray(p, dtype=np.float64)
        for p in range(SEGP):
            seg += S[p * NM:(p + 1) * NM, p * C:(p + 1) * C]

    labels = np.asarray(labels).astype(np.int64)
    label2 = np.asarray(label2)
    Ci = int(num_classes)
    w = (label2 == 1)
    cnt = np.bincount(labels[w], minlength=Ci).astype(np.float64)[:C]
    present = np.bincount(labels, minlength=Ci)[:C] > 0

    safe = np.maximum(cnt, 1.0)
    meand = seg[0:K] / safe[None, :]          # [K, C]
    closest = np.argmin(meand, axis=0)        # [C]
    # sum_c viol = sum v1 + [sum mask_j*(v2-v1)] at the class's closest j
    sv = seg[K] + seg[K + 1 + closest, np.arange(C)]
    has = (cnt > 0).astype(np.float64)
    per_class = (seg[11] + sv) / safe * has
    n_unique = max(float(present.sum()), 1.0)
    return np.float32(per_class.sum() / n_unique)


_NC_CACHE = {}


def kernel(feat_normed, labels, label2, num_classes, centers, _trace=False):
    label2 = np.asarray(label2)
    nw = int((label2 == 1).sum())
    if nw == 0:
        return np.asarray(
            _host_final([], labels, label2, num_classes), dtype=np.float32
        )
    feat_normed = np.asarray(feat_normed, dtype=np.float32)
    # unit-norm fast path (feat_normed is normalized by construction);
    # sampled check with a per-row-|x|^2-table fallback build
    samp = feat_normed[:: max(1, feat_normed.shape[0] // 512)]
    unit_norm = bool(
        np.allclose(np.einsum("ij,ij->i", samp, samp), 1.0, atol=1e-3)
    )
    tiles = -(-nw // (128 * NCORES))          # ceil rows / (128*cores)
    tiles = SEGP * (-(-tiles // SEGP))        # round up to multiple of SEGP
    key = (tiles, unit_norm)
    if key not in _NC_CACHE:
        _NC_CACHE[key] = build_nc(tiles, unit_norm)
    nc = _NC_CACHE[key]
    in_maps = _host_prep(
        feat_normed, labels, label2, centers, tiles, unit_norm
    )
    res = run_bass_kernel_spmd(
        nc, in_maps, core_ids=list(range(NCORES)), trace=_trace
    )
    parts = [r["out"] for r in res.results]
    out = _host_final(parts, labels, label2, num_classes)
    if _trace:
        kernel.last_result = res
    return np.asarray(out, dtype=np.float32)
